# revision 38
# baseline (speedup 1.0000x reference)
"""Trainium2 Bass kernel for the lipsnet CustomModel problem.

Math: the reference computes, per sample,
    jac_norm = ||D3 W3 D2 W2 D1 W1||_F      (Di = diag(relu'(pi)))
    out = tanh(k_out * f_out / (jac_norm + 1e-4))
Key identity:  with G = W1 W1^T = L L^T (host eigen factorization),
    ||D3 W3 D2 W2 D1 W1||_F^2 = sum_c || D3 W3 D2 (M_c @ d1) ||^2
where M_c[j,l] = W2[j,l] * L[l,c] are 85 host-precomputed stationary
matrices and d1/d2/d3 are the per-sample binary relu masks.  Every
per-sample 85x85x85 contraction becomes a stationary-weight matmul with
the mask tensor as the moving operand.

This version vs the original baseline:
  * all forward matmuls in bf16 (1 cycle/row instead of 4 for fp32)
  * constant weights are DMA'd once, outside the rep loop
  * d-mask production and the final d3-mask multiply run on the (idle)
    GpSimd engine to keep DVE free for the per-c mask multiplies
  * k_out and jn2 are produced pre-transposed via 1-column matmuls
    (lhsT = data block), so no [1,S]-wide ACT/DVE passes exist
  * fout^T is staged during the forward pass; the finale is just
    sqrt/recip/mul on [128,8] tiles + 8 tanh+DMA blocks

Sharding: pure data parallel over the batch dim, 8 NeuronCores, weights
replicated.  kernel() takes FULL inputs and returns the FULL output.
"""

from contextlib import ExitStack

import numpy as np

import concourse.bass as bass
import concourse.bacc as bacc
import concourse.mybir as mybir
import concourse.tile as tile

F32 = mybir.dt.float32
BF16 = mybir.dt.bfloat16
F8 = mybir.dt.float8e4  # unused currently (DoubleRow needs k>128 to pay off)
AF = mybir.ActivationFunctionType
OP = mybir.AluOpType
DR = mybir.MatmulPerfMode.DoubleRow

B = 8192
OBS = 64
ACTD = 16
H = 128
COMP = 85
KS = 32
NCORES = 8
S = B // NCORES        # 1024 samples per core
NB = S // 128          # 8 sample blocks of 128
CH = 512               # matmul moving-operand chunk (one PSUM bank of f32)
EPS = 1e-4
GRP = 28               # PSUM accumulation-group length for the c-accumulate

# ---- packed replicated weights: (shape, pack) with pack 0=f32 1=bf16 ----
_WSLOTS = {
    # f32: biases (per-partition scalars), transpose identity
    "ob1": ([H, 1], 0), "ob2": ([H, 1], 0),
    "ab1": ([H, 1], 0), "ab2": ([H, 1], 0),
    "kb1": ([KS, 1], 0), "kb2": ([KS // 2, 1], 0), "kb3r": ([128, 1], 0),
    "mb1": ([COMP, 1], 0), "mb2": ([COMP, 1], 0), "mb3": ([COMP, 1], 0),
    "iden": ([H, H], 0),
    # bf16: all matmul stationaries
    "ow1T": ([OBS, H], 1), "ow2T": ([H, H], 1),
    "aw1T": ([ACTD, H], 1), "aw2T": ([H, H], 1),
    "kw1Ta": ([H, KS], 1), "kw1Tb": ([H, KS], 1),
    "kw2T": ([KS, KS // 2], 1), "kw3T": ([KS // 2, 1], 1),
    "mw1Ta": ([H, COMP], 1), "mw1Tb": ([H, COMP], 1),
    "mw2T": ([COMP, COMP], 1),
    "mw3T": ([COMP, COMP], 1),
    "idenb": ([COMP, COMP], 1), "onesb": ([COMP, 1], 1),
    "idenc": ([H, H], 1),
    "mall": ([COMP, COMP * COMP], 1),
}
_OFFS = {}
_NCOLS = [0, 0]
for _n, (_shp, _b) in _WSLOTS.items():
    _OFFS[_n] = _NCOLS[_b]
    _NCOLS[_b] += _shp[1]


def host_prep(inputs):
    """Host-side weight preprocessing + packing (pure numpy, all tiny)."""
    f = lambda a: np.ascontiguousarray(np.asarray(a, dtype=np.float32))
    W1, W2, W3 = f(inputs["mw1"]), f(inputs["mw2"]), f(inputs["mw3"])
    G = (W1 @ W1.T).astype(np.float64)
    lam, U = np.linalg.eigh(G)
    L = (U * np.sqrt(np.clip(lam, 0.0, None))).astype(np.float32)  # G = L L^T

    # mall[j, c*85+m] = W2[m, j] * L[j, c]   (stage-1 stationary lhsT per c)
    mall = (W2.T[:, None, :] * L[:, :, None]).reshape(COMP, COMP * COMP)

    bf = mybir.dt.np(BF16)
    vals32 = {
        "ob1": f(inputs["ob1"]).reshape(H, 1), "ob2": f(inputs["ob2"]).reshape(H, 1),
        "ab1": f(inputs["ab1"]).reshape(H, 1), "ab2": f(inputs["ab2"]).reshape(H, 1),
        "kb1": f(inputs["kb1"]).reshape(KS, 1),
        "kb2": f(inputs["kb2"]).reshape(KS // 2, 1),
        "kb3r": np.full((128, 1), float(np.asarray(inputs["kb3"]).reshape(-1)[0]),
                        np.float32),
        "mb1": f(inputs["mb1"]).reshape(COMP, 1),
        "mb2": f(inputs["mb2"]).reshape(COMP, 1),
        "mb3": f(inputs["mb3"]).reshape(COMP, 1),
        "iden": np.eye(H, dtype=np.float32),
    }
    vals16 = {
        "ow1T": f(inputs["ow1"]).T, "ow2T": f(inputs["ow2"]).T,
        "aw1T": f(inputs["aw1"]).T, "aw2T": f(inputs["aw2"]).T,
        "kw1Ta": f(inputs["kw1"]).T[:H], "kw1Tb": f(inputs["kw1"]).T[H:],
        "kw2T": f(inputs["kw2"]).T, "kw3T": f(inputs["kw3"]).T,
        "mw1Ta": W1.T[:H], "mw1Tb": W1.T[H:],
        "mw2T": W2.T, "mw3T": W3.T,
        "idenb": np.eye(COMP, dtype=np.float32),
        "onesb": np.ones((COMP, 1), np.float32),
        "idenc": np.eye(H, dtype=np.float32),
        "mall": mall,
    }
    packs = [np.zeros((128, _NCOLS[0]), np.float32),
             np.zeros((128, _NCOLS[1]), bf)]
    for n, (shp, b) in _WSLOTS.items():
        o = _OFFS[n]
        packs[b][: shp[0], o : o + shp[1]] = (vals32 if b == 0 else vals16)[n]
    return {"wpack32": packs[0], "wpack16": packs[1]}


def build_nc(reps=1, ncomp=COMP, jmode="full", accdve=45):
    # ncomp < COMP builds a timing variant with a truncated J-loop (output
    # values are then wrong; only used by bench scripts for differencing).
    # jmode: "full" = real kernel; "nodep" = z reads an SBUF tile (breaks
    # the py->z dependency, wrong output); "peonly" = no per-c DVE/ACT ops
    # at all (PE stream only, wrong output). Timing probes.
    # accdve: how many of the ncomp square-accumulate adds run on DVE; the
    # rest run on GpSimd. (The accumulate used to be an identity matmul on
    # the PE, but per-instruction overhead makes PE issue slots precious.)
    nc = bacc.Bacc()

    obs_d = nc.declare_dram_parameter("obs", [S, OBS], F32, isOutput=False)
    act_d = nc.declare_dram_parameter("action", [S, ACTD], F32, isOutput=False)
    wp32_d = nc.declare_dram_parameter("wpack32", [128, _NCOLS[0]], F32,
                                       isOutput=False)
    wp16_d = nc.declare_dram_parameter("wpack16", [128, _NCOLS[1]], BF16,
                                       isOutput=False)
    out_d = nc.declare_dram_parameter("out", [S, COMP], F32, isOutput=True)

    with tile.TileContext(nc) as tc, ExitStack() as ctx:
        wp = ctx.enter_context(tc.tile_pool(name="weights", bufs=1))
        inp = ctx.enter_context(tc.tile_pool(name="inbuf", bufs=2))
        ap = ctx.enter_context(tc.tile_pool(name="acts", bufs=1))
        zp = ctx.enter_context(tc.tile_pool(name="zbuf", bufs=4))
        sqp = ctx.enter_context(tc.tile_pool(name="sqbuf", bufs=4))
        outp = ctx.enter_context(tc.tile_pool(name="outbuf", bufs=3))
        smp = ctx.enter_context(tc.tile_pool(name="small", bufs=16))
        psA = ctx.enter_context(tc.tile_pool(name="psA", bufs=4, space="PSUM"))

        # ---- load weights once (2 packed DMAs), expose slice views ----
        wp32 = wp.tile([128, _NCOLS[0]], F32, tag="wp32", name="wp32")
        wp16 = wp.tile([128, _NCOLS[1]], BF16, tag="wp16", name="wp16")
        nc.sync.dma_start(wp32[:], wp32_d[:])
        nc.sync.dma_start(wp16[:], wp16_d[:])
        w = {}
        for name, (shp, b) in _WSLOTS.items():
            o = _OFFS[name]
            w[name] = (wp16 if b else wp32)[0 : shp[0], o : o + shp[1]]

        for _rep in range(reps):
            # ---- load obs/action, transpose into [feat, S] bf16 ----
            obs_sb = inp.tile([128, NB, OBS], F32, tag="obs_sb")
            act_sb = inp.tile([128, NB, ACTD], F32, tag="act_sb")
            for nb in range(NB):
                nc.sync.dma_start(obs_sb[:, nb, :], obs_d[nb * 128:(nb + 1) * 128, :])
                nc.sync.dma_start(act_sb[:, nb, :], act_d[nb * 128:(nb + 1) * 128, :])
            if _rep == 0:
                # one barrier after the initial loads collapses the weight +
                # input DMA semaphores; later reps overlap their input DMA
                # with the previous rep's J-loop (inp pool is double-buffered)
                tc.strict_bb_all_engine_barrier()

            obst = ap.tile([OBS, S], BF16, tag="obst")
            actt = ap.tile([ACTD, S], BF16, tag="actt")
            pto = psA.tile([OBS, 2, NB // 2, 128], F32, tag="a", name="pto")
            pta = psA.tile([ACTD, NB, 128], F32, tag="a", name="pta")
            for nb in range(NB):
                nc.tensor.transpose(pto[:, nb // (NB // 2), nb % (NB // 2), :],
                                    obs_sb[:, nb, :], w["iden"][:])
                nc.tensor.transpose(pta[:, nb, :], act_sb[:, nb, :], w["iden"][:])
            nc.scalar.copy(obst[:].rearrange("f (t h s) -> f t h s", t=2, h=NB // 2),
                           pto[:])
            nc.scalar.copy(actt[:].rearrange("f (h s) -> f h s", h=NB), pta[:])

            # ---- forward layers ([feat, S], chunked bf16 matmuls) ----
            def layer(dst, terms, bias, func, eng="v"):
                # dst = func(sum_i lhsT_i.T @ rhs_i + bias); matmuls write a
                # full-width 2-bank PSUM tile, one activation op consumes it
                pt = psA.tile([terms[0][0].shape[-1], S], F32, tag="a",
                              name="pt")
                n = len(terms)
                for ch in range(S // CH):
                    sl = slice(ch * CH, (ch + 1) * CH)
                    for i, (lhsT, rhs) in enumerate(terms):
                        nc.tensor.matmul(pt[:, sl], lhsT[:], rhs[:, sl],
                                         start=(i == 0), stop=(i == n - 1))
                if func == AF.Relu and eng == "v":
                    nc.vector.tensor_scalar(out=dst[:], in0=pt[:],
                                            scalar1=bias[:], scalar2=0.0,
                                            op0=OP.add, op1=OP.max)
                else:
                    nc.scalar.activation(dst[:], pt[:], func, bias=bias[:])

            oh1 = ap.tile([H, S], BF16, tag="oh1")
            layer(oh1, [(w["ow1T"], obst)], w["ob1"], AF.Relu)
            of = ap.tile([H, S], BF16, tag="of")
            layer(of, [(w["ow2T"], oh1)], w["ob2"], AF.Relu)
            ah1 = ap.tile([H, S], BF16, tag="ah1")
            layer(ah1, [(w["aw1T"], actt)], w["ab1"], AF.Relu)
            af = ap.tile([H, S], BF16, tag="af")
            layer(af, [(w["aw2T"], ah1)], w["ab2"], AF.Relu)

            k1 = ap.tile([KS, S], BF16, tag="k1")
            layer(k1, [(w["kw1Ta"], of), (w["kw1Tb"], af)], w["kb1"], AF.Tanh,
                  eng="s")
            k2 = ap.tile([KS // 2, S], BF16, tag="k2")
            layer(k2, [(w["kw2T"], k1)], w["kb2"], AF.Tanh, eng="s")
            # k_out, pre-transposed: per 128-block, kpreT[s,1] = k2[:,blk]^T kw3T
            # via a 1-column matmul; then softplus on the narrow [128, NB] tile
            pk = psA.tile([128, NB], F32, tag="a", name="pk")
            for nb in range(NB):
                nc.tensor.matmul(pk[:, nb : nb + 1],
                                 k2[:, nb * 128:(nb + 1) * 128], w["kw3T"][:],
                                 start=True, stop=True)
            kex = smp.tile([128, NB], F32, tag="kex")
            nc.scalar.activation(kex[:], pk[:], AF.Exp, bias=w["kb3r"][:])
            kout = smp.tile([128, NB], F32, tag="kout")
            nc.scalar.activation(kout[:], kex[:], AF.Ln, bias=1.0)

            h1 = ap.tile([COMP, S], BF16, tag="h1")
            layer(h1, [(w["mw1Ta"], of), (w["mw1Tb"], af)], w["mb1"], AF.Relu)
            d1q = ap.tile([COMP, S], BF16, tag="d1q")
            nc.gpsimd.tensor_scalar(out=d1q[:], in0=h1[:], scalar1=0.0,
                                    scalar2=None, op0=OP.is_gt)

            h2 = ap.tile([COMP, S], BF16, tag="h2")
            layer(h2, [(w["mw2T"], h1)], w["mb2"], AF.Relu)
            d2 = ap.tile([COMP, S], BF16, tag="d2")
            nc.gpsimd.tensor_scalar(out=d2[:], in0=h2[:], scalar1=0.0,
                                    scalar2=None, op0=OP.is_gt)
            fout = ap.tile([COMP, S], BF16, tag="fout")
            layer(fout, [(w["mw3T"], h2)], w["mb3"], AF.Relu, eng="s")
            d3 = ap.tile([COMP, S], BF16, tag="d3")
            nc.gpsimd.tensor_scalar(out=d3[:], in0=fout[:], scalar1=0.0,
                                    scalar2=None, op0=OP.is_gt)
            # stage fout^T per 128-block now, while PSUM/ACT are free; the
            # finale then only needs the scale + tanh + DMA
            foutT = ap.tile([128, NB, COMP], BF16, tag="foutT")
            for g in range(2):
                # 86-wide block stride keeps each bf16 PSUM offset 4B-aligned
                ptf = psA.tile([128, NB // 2, COMP + 1], BF16, tag="a",
                               name="ptf")
                for i in range(NB // 2):
                    nb = g * (NB // 2) + i
                    nc.tensor.transpose(ptf[:, i, :COMP],
                                        fout[:, nb * 128:(nb + 1) * 128],
                                        w["idenc"][:COMP, :COMP])
                nc.scalar.copy(foutT[:, g * (NB // 2):(g + 1) * (NB // 2), :],
                               ptf[:, :, :COMP])

            # ---- Jacobian-norm loop over the 85 columns of L ----
            # per-engine square accumulators: sum_c sq_c lands in ACCd (DVE
            # adds) + ACCp (GpSimd adds), merged once at the end.  This keeps
            # every PE issue slot for the py/pr matmuls.
            ACCd = ap.tile([COMP, S], F32, tag="ACCd")
            ACCp = ap.tile([COMP, S], F32, tag="ACCp")
            nd = max(0, min(accdve, ncomp))
            on_dve = [(n * nd) // ncomp != ((n + 1) * nd) // ncomp
                      for n in range(ncomp)]
            acc_state = {"n": 0, "d": False, "p": False}

            def acc_add(sq):
                n = acc_state["n"]
                acc_state["n"] = n + 1
                if on_dve[n]:
                    eng, acc, key = nc.vector, ACCd, "d"
                else:
                    eng, acc, key = nc.gpsimd, ACCp, "p"
                if not acc_state[key]:
                    acc_state[key] = True
                    eng.tensor_copy(acc[:], sq[:])
                else:
                    eng.tensor_tensor(acc[:], sq[:], acc[:], OP.add)

            # software pipeline, PE-continuity-first: py is prefetched TWO c
            # ahead and the PE stream per iteration is [py(c+2), acc(c-2),
            # pr(c)] — every cross-engine dependency has >= 1 full iteration
            # of slack, so the PE never waits mid-loop (psA allocation order
            # is chosen so each buffer-reuse WAR edge coincides with an
            # already-satisfied true dependency).
            pys = {}

            def emit_py(c):
                t = psA.tile([COMP, S], F32, tag="a", name="py")
                for ch in range(S // CH):
                    sl = slice(ch * CH, (ch + 1) * CH)
                    nc.tensor.matmul(t[:, sl],
                                     w["mall"][:, c * COMP:(c + 1) * COMP],
                                     d1q[:, sl], start=True, stop=True)
                pys[c] = t

            emit_py(0)
            if ncomp > 1:
                emit_py(1)
            pend = []
            for c in range(ncomp):
                pr = psA.tile([COMP, S], F32, tag="a", name="pr")
                if c + 2 < ncomp:
                    emit_py(c + 2)
                if len(pend) == 2:
                    acc_add(pend.pop(0))
                if jmode == "peonly":
                    pys.pop(c)
                    for ch in range(S // CH):
                        sl = slice(ch * CH, (ch + 1) * CH)
                        nc.tensor.matmul(pr[:, sl], w["mw3T"][:], d2[:, sl],
                                         start=True, stop=True)
                    sq = d1q
                else:
                    z = zp.tile([COMP, S], BF16, tag="z")
                    src = d3 if jmode == "nodep" else pys[c]
                    pys.pop(c)
                    nc.vector.tensor_tensor(z[:], src[:], d2[:], OP.mult)
                    for ch in range(S // CH):
                        sl = slice(ch * CH, (ch + 1) * CH)
                        nc.tensor.matmul(pr[:, sl], w["mw3T"][:], z[:, sl],
                                         start=True, stop=True)
                    sq = sqp.tile([COMP, S], BF16, tag="sq")
                    nc.scalar.square(sq[:], pr[:])
                pend.append(sq)
            acc_add(pend.pop(0))
            acc_add(pend.pop(0))

            # ---- finale: jn2^T per block via 1-column matmuls, then the
            # scale chain on narrow [128, NB] tiles, tanh, and DMA out ----
            if acc_state["d"] and acc_state["p"]:
                ACCm = ACCd
                nc.vector.tensor_tensor(ACCm[:], ACCp[:], ACCd[:], OP.add)
            else:
                ACCm = ACCd if acc_state["d"] else ACCp
            am = zp.tile([COMP, S], BF16, tag="am")
            nc.gpsimd.tensor_tensor(am[:], ACCm[:], d3[:], OP.mult)
            pjq = psA.tile([128, NB], F32, tag="a", name="pjq")
            for nb in range(NB):
                nc.tensor.matmul(pjq[:, nb : nb + 1],
                                 am[:, nb * 128:(nb + 1) * 128], w["onesb"][:],
                                 start=True, stop=True)
            # den = sqrt(jn2) + EPS ; scl = kout/den
            den = smp.tile([128, NB], F32, tag="den")
            nc.scalar.activation(den[:], pjq[:], AF.Sqrt)
            rec = smp.tile([128, NB], F32, tag="rec")
            nc.vector.tensor_scalar_add(rec[:], den[:], EPS)
            nc.vector.reciprocal(rec[:], rec[:])
            scl = smp.tile([128, NB], F32, tag="scl")
            nc.vector.tensor_tensor(scl[:], rec[:], kout[:], OP.mult)
            for nb in range(NB):
                ot = outp.tile([128, COMP], F32, tag="ot")
                nc.scalar.activation(ot[:], foutT[:, nb, :], AF.Tanh,
                                     scale=scl[:, nb : nb + 1])
                nc.sync.dma_start(out_d[nb * 128:(nb + 1) * 128, :], ot[:])

    return nc


_NC = None


def _get_nc():
    global _NC
    if _NC is None:
        _NC = build_nc()
        _NC.finalize()
    return _NC


def make_in_maps(inputs):
    w = host_prep(inputs)
    obs = np.ascontiguousarray(np.asarray(inputs["obs"], np.float32))
    act = np.ascontiguousarray(np.asarray(inputs["action"], np.float32))
    in_maps = []
    for i in range(NCORES):
        m = dict(w)
        m["obs"] = np.ascontiguousarray(obs[i * S:(i + 1) * S])
        m["action"] = np.ascontiguousarray(act[i * S:(i + 1) * S])
        in_maps.append(m)
    return in_maps


def kernel(**inputs):
    from concourse.bass_utils import run_bass_kernel_spmd

    nc = _get_nc()
    in_maps = make_in_maps(inputs)
    res = run_bass_kernel_spmd(nc, in_maps, core_ids=list(range(NCORES)))
    return np.concatenate([r["out"] for r in res.results], axis=0)
